# revision 7
# baseline (speedup 1.0000x reference)
"""CrossAttention Trainium2 Bass kernel — 8 cores, batch-per-core sharding.

Per core b: all H=8 heads of batch b, processed as 4 passes of
(head-QUAD x q-half).  Within a pass, per kt:
  sT[k, (h4, q512)] = k @ qT    4 heads row-strip packed (strips 0/32/64/96)
  es = exp(sT)                  ONE activation over [128, 2048]
  p  = es * exp(bias)           host-precomputed eb fp16 (scaled 1/16)
  waT += v' @ p                 4 heads col-strip packed into 2 psum banks

The PE stream is software-pipelined with a 2-step skew (qk(kt) issues
before pv(kt-2)) so mul/pv latency never blocks the exp->qk->exp chain,
which paces the kernel at ~2.8us per kt (scalar exp 1.97us + qk WAR).
Finalize (cast fp16, PE-transpose, recip(den), out = waT*r*gate) is
interleaved into the next pass.  gate = sigmoid(q_data @ Wg) runs first
so the ACT table set switches exactly once (sigmoid set -> exp set).
Half-1 q/k projections run as 1-bank PSUM installments during pass 1.

PSUM (8 banks): logits 4 + pv accum 2 + transpose staging 1 + gate/proj 1.
exp(bias) rides two DMA queues (sync + gpsimd) as contiguous 1MB chunks.
"""
import numpy as np
from contextlib import ExitStack

import concourse.bass as bass
import concourse.tile as tile
from concourse import mybir
from concourse.bass import AP
from concourse.bass_utils import run_bass_kernel_spmd
from concourse.masks import make_identity

F32 = mybir.dt.float32
F16 = mybir.dt.float16

B, S, K, H, C, V, A = 8, 1024, 1024, 8, 32, 32, 256
HV = H * V            # 256
KEY_SCALE = C ** -0.5
N_CORES = 8
QT = S // 128         # 8 q tiles
KT = K // 128         # 8 k tiles
NPASS = 4             # (quad, q-half) passes
NCHUNK = 16           # eb chunks of [128, 2, 2048]
EB_SCALE = 1.0 / 16.0  # host scales exp(bias); cancels in softmax


def _split_multi_waits(nc, max_waits=1):
    """walrus in this container allows only one semaphore wait per
    instruction; hoist extras onto same-engine nops inserted just before."""
    ctr = 0
    for fn in nc.m.functions:
        for blk in fn.blocks:
            insts = list(blk.instructions)
            out = []
            changed = False
            for inst in insts:
                si = inst.sync_info
                waits = list(si.on_wait) if (si is not None and si.on_wait) else []
                if len(waits) > max_waits:
                    changed = True
                    extra, keep = waits[:-max_waits], waits[-max_waits:]
                    for w in extra:
                        ctr += 1
                        nop = mybir.InstNoOp(
                            name=f"waitsplit_{ctr}",
                            engine=inst.engine,
                            ins=[],
                            outs=[],
                            sync_info=mybir.SyncInfo(on_wait=[w], on_update=[]),
                            bass_nofuse=True,
                        )
                        out.append(nop)
                    si.on_wait = keep
                out.append(inst)
            if changed:
                if hasattr(blk, "set_instructions"):
                    blk.set_instructions(out)
                else:
                    blk.instructions = out
    return ctr


def build():
    nc = bass.Bass()
    qT_d = nc.declare_dram_parameter("qT", [A, S], F16, isOutput=False)
    mT_d = nc.declare_dram_parameter("mT", [A, K], F16, isOutput=False)
    expb_d = nc.declare_dram_parameter("expb", [NPASS, KT, 128, 2048], F16,
                                       isOutput=False)
    wq_d = nc.declare_dram_parameter("wq", [A, HV], F16, isOutput=False)
    wk_d = nc.declare_dram_parameter("wk", [A, HV], F16, isOutput=False)
    wv_d = nc.declare_dram_parameter("wv", [A, HV], F16, isOutput=False)
    wg_d = nc.declare_dram_parameter("wg", [A, HV], F16, isOutput=False)
    bq_d = nc.declare_dram_parameter("bq", [HV], F32, isOutput=False)
    out_d = nc.declare_dram_parameter("out", [S, HV], F32, isOutput=True)

    with tile.TileContext(nc) as tc, ExitStack() as ctx:
        singles = ctx.enter_context(tc.tile_pool(name="singles", bufs=1))
        eb_pool = ctx.enter_context(tc.tile_pool(name="eb", bufs=4))
        es_pool = ctx.enter_context(tc.tile_pool(name="es", bufs=3))
        p_pool = ctx.enter_context(tc.tile_pool(name="pp", bufs=3))
        fin_pool = ctx.enter_context(tc.tile_pool(name="fin", bufs=2))
        dr_pool = ctx.enter_context(tc.tile_pool(name="dr", bufs=2))
        rg_pool = ctx.enter_context(tc.tile_pool(name="rg", bufs=2))
        psum = ctx.enter_context(tc.tile_pool(name="ps", bufs=1, space="PSUM"))

        # ---------- phase 0: loads split across both DMA queues ------------
        qraw = singles.tile([128, 2, S], F16)       # [a-chunk part, chunk, q]
        mraw = singles.tile([128, 2, K], F16)
        wq_sb = singles.tile([128, 2, HV], F16)
        wk_sb = singles.tile([128, 2, HV], F16)
        wv_sb = singles.tile([128, 2, HV], F16)
        wg_sb = singles.tile([128, 2, HV], F16)
        bq_sb = singles.tile([128, 2], F32)
        # sync queue: gate+q-proj inputs first
        for ac in range(2):
            nc.sync.dma_start(out=qraw[:, ac, :], in_=qT_d[ac * 128:(ac + 1) * 128, :])
        for ac in range(2):
            nc.sync.dma_start(out=wg_sb[:, ac, :], in_=wg_d[ac * 128:(ac + 1) * 128, :])
        for ac in range(2):
            nc.sync.dma_start(out=wq_sb[:, ac, :], in_=wq_d[ac * 128:(ac + 1) * 128, :])
        nc.sync.dma_start(out=bq_sb, in_=bq_d.rearrange("(h p) -> p h", p=128))
        # gpsimd queue: k/v-proj inputs, then the eb chunk stream
        for ac in range(2):
            nc.gpsimd.dma_start(out=mraw[:, ac, :], in_=mT_d[ac * 128:(ac + 1) * 128, :])
        for ac in range(2):
            nc.gpsimd.dma_start(out=wk_sb[:, ac, :], in_=wk_d[ac * 128:(ac + 1) * 128, :])
        for ac in range(2):
            nc.gpsimd.dma_start(out=wv_sb[:, ac, :], in_=wv_d[ac * 128:(ac + 1) * 128, :])
        ident = singles.tile([128, 128], F16)
        make_identity(nc, ident)
        v_sb = singles.tile([128, KT, H, V + 1], F16)
        nc.gpsimd.memset(v_sb, 1.0)

        # eb chunks: chunk g covers (pass g//4, kts 2*(g%4)..2*(g%4)+1)
        eb_tiles = {}

        def fetch_eb(g):
            ps, c = g // 4, g % 4
            t = eb_pool.tile([128, 2, 2048], F16, tag="eb", name=f"eb{ps}_{c}")
            eng = nc.gpsimd if (g < 3 or g % 2 == 0) else nc.sync
            eng.dma_start(
                out=t,
                in_=expb_d[ps, 2 * c:2 * c + 2, :, :].rearrange("k p f -> p k f"))
            eb_tiles[g] = t

        for g in range(4):
            fetch_eb(g)

        # ---------- phase 1: gate + half-0 projections ----------
        # gate first: its sigmoids run before any exp -> one table switch.
        gate_sb = singles.tile([128, QT, HV], F16)
        for grp in range(4):         # qt groups of 2, 1 psum bank each
            ps_g = psum.tile([128, 2, HV], F32, tag="g", name=f"ps_projg{grp}")
            for qq in range(2):
                qt = grp * 2 + qq
                for ac in range(2):
                    nc.tensor.matmul(ps_g[:, qq, :],
                                     lhsT=qraw[:, ac, qt * 128:(qt + 1) * 128],
                                     rhs=wg_sb[:, ac, :], start=(ac == 0),
                                     stop=(ac == 1))
            nc.scalar.activation(gate_sb[:, grp * 2:(grp + 1) * 2, :], ps_g,
                                 mybir.ActivationFunctionType.Sigmoid)

        qT_sb = singles.tile([128, 2, S], F16)
        kT_sb = singles.tile([128, 2, K], F16)

        def emit_qproj(half, qh, ptag):
            t = psum.tile([128, 512] if ptag == "g" else [128, 2, 512],
                          F32, tag=ptag, name=f"q{half}_{qh}")
            dst = t if ptag == "g" else t[:, qh, :]
            for ac in range(2):
                nc.tensor.matmul(dst if ptag == "g" else t[:, qh, :],
                                 lhsT=wq_sb[:, ac, half * 128:(half + 1) * 128],
                                 rhs=qraw[:, ac, qh * 512:(qh + 1) * 512],
                                 start=(ac == 0), stop=(ac == 1))
            nc.vector.tensor_scalar(
                qT_sb[:, half, qh * 512:(qh + 1) * 512],
                dst, KEY_SCALE, bq_sb[:, half:half + 1],
                mybir.AluOpType.mult, mybir.AluOpType.add)

        def emit_kproj(half, qh, ptag):
            t = psum.tile([128, 512] if ptag == "g" else [128, 2, 512],
                          F32, tag=ptag, name=f"k{half}_{qh}")
            dst = t if ptag == "g" else t[:, qh, :]
            for ac in range(2):
                nc.tensor.matmul(dst,
                                 lhsT=wk_sb[:, ac, half * 128:(half + 1) * 128],
                                 rhs=mraw[:, ac, qh * 512:(qh + 1) * 512],
                                 start=(ac == 0), stop=(ac == 1))
            nc.vector.tensor_copy(out=kT_sb[:, half, qh * 512:(qh + 1) * 512],
                                  in_=dst)

        # half 0 up-front: q on the "s" slot, k on the "wa" slot (decoupled)
        ps_q0 = psum.tile([128, 2, 512], F32, tag="s", name="ps_q0")
        for qh in range(2):
            for ac in range(2):
                nc.tensor.matmul(ps_q0[:, qh, :],
                                 lhsT=wq_sb[:, ac, 0:128],
                                 rhs=qraw[:, ac, qh * 512:(qh + 1) * 512],
                                 start=(ac == 0), stop=(ac == 1))
        nc.vector.tensor_scalar(
            qT_sb[:, 0, :], ps_q0.rearrange("p a f -> p (a f)"),
            KEY_SCALE, bq_sb[:, 0:1],
            mybir.AluOpType.mult, mybir.AluOpType.add)
        ps_k0 = psum.tile([128, 2, 512], F32, tag="wa", name="ps_k0")
        for qh in range(2):
            for ac in range(2):
                nc.tensor.matmul(ps_k0[:, qh, :],
                                 lhsT=wk_sb[:, ac, 0:128],
                                 rhs=mraw[:, ac, qh * 512:(qh + 1) * 512],
                                 start=(ac == 0), stop=(ac == 1))
        nc.vector.tensor_copy(out=kT_sb[:, 0, :],
                              in_=ps_k0.rearrange("p a f -> p (a f)"))

        # v natural layout + ones column: [k-tile part, kt, h, v+1] fp16
        def emit_vproj(grp):
            ps_v = psum.tile([128, 4, HV], F32, tag="wa", name=f"ps_projv{grp}")
            for kq in range(4):
                kt = grp * 4 + kq
                for ac in range(2):
                    nc.tensor.matmul(ps_v[:, kq, :],
                                     lhsT=mraw[:, ac, kt * 128:(kt + 1) * 128],
                                     rhs=wv_sb[:, ac, :], start=(ac == 0),
                                     stop=(ac == 1))
            nc.vector.tensor_copy(
                out=v_sb[:, grp * 4:(grp + 1) * 4, :, 0:V],
                in_=ps_v.rearrange("p k (h c) -> p k h c", c=V))

        emit_vproj(0)
        emit_vproj(1)

        # ---------- phase 2: quad passes, 2-step qk/pv skew ----------
        out_sb = singles.tile([128, QT, HV], F32)
        pending_fin = []   # finalize steps of the previous pass
        pv_queue = []      # deferred pv emitters

        def make_finalizer(ps_idx, wa):
            quad, qhalf = ps_idx // 2, ps_idx % 2
            fin = fin_pool.tile([128, 1024], F16, tag="fin", name=f"fin{ps_idx}")
            ps_t = psum.tile([128, 8, 128], F16, tag="t", name=f"pst{ps_idx}")
            steps = [lambda: nc.vector.tensor_copy(
                out=fin.rearrange("p (k f) -> p k f", k=2), in_=wa)]
            for ch in range(8):      # chunk = pi*4 + j
                steps.append(lambda ch=ch: nc.tensor.transpose(
                    ps_t[:, ch, :], fin[:, ch * 128:(ch + 1) * 128], ident))

            def tail():
                # den at ps_t[:, pi*4+j, 32 + 64*hh] -> d[128, j, head(pi,hh)]
                d_src = AP(ps_t.tensor, ps_t[:, 0, 32].offset,
                           [list(ps_t.ap)[0], [128, 4], [512, 2], [64, 2]])
                d_sb = dr_pool.tile([128, 4, 4], F32, tag="d", name=f"d{ps_idx}")
                nc.vector.tensor_copy(out=d_sb, in_=d_src)
                r_sb = dr_pool.tile([128, 4, 4], F32, tag="r", name=f"r{ps_idx}")
                nc.vector.reciprocal(out=r_sb, in_=d_sb)
                # rg[q, j, head, v] = gate * r (r broadcast over v, stride 0)
                r_b = AP(r_sb.tensor, r_sb.offset,
                         [list(r_sb.ap)[0], [4, 4], [1, 4], [0, V]])
                rg = rg_pool.tile([128, 4, 4, V], F32, tag="rg",
                                  name=f"rg{ps_idx}")
                nc.vector.tensor_mul(
                    out=rg,
                    in0=gate_sb[:, qhalf * 4:(qhalf + 1) * 4,
                                quad * 128:(quad + 1) * 128].rearrange(
                        "p j (h v) -> p j h v", v=V),
                    in1=r_b)
                # out[q, j, quad cols] = waT * rg   (one op per pair pi)
                for pi in range(2):
                    src = AP(ps_t.tensor, ps_t[:, pi * 4, 0].offset,
                             [list(ps_t.ap)[0], [128, 4], [64, 2], [1, V]])
                    nc.vector.tensor_mul(
                        out=out_sb[:, qhalf * 4:(qhalf + 1) * 4,
                                   quad * 128 + pi * 64:
                                   quad * 128 + (pi + 1) * 64].rearrange(
                            "p j (k v) -> p j k v", v=V),
                        in0=src,
                        in1=rg[:, :, pi * 2:(pi + 1) * 2, :])

            def store():
                for j in range(4):
                    qt = qhalf * 4 + j
                    nc.sync.dma_start(
                        out=out_d[qt * 128:(qt + 1) * 128,
                                  quad * 128:(quad + 1) * 128],
                        in_=out_sb[:, qt, quad * 128:(quad + 1) * 128])
            steps.append(tail)
            steps.append(store)
            return steps

        # half-1 projection installments, spread through pass 1 (g slot)
        proj1 = [lambda qh=qh: emit_qproj(1, qh, "g") for qh in range(2)]
        proj1 += [lambda qh=qh: emit_kproj(1, qh, "g") for qh in range(2)]

        for ps_idx in range(NPASS):
            quad, qhalf = ps_idx // 2, ps_idx % 2
            half = quad
            wa = psum.tile([128, 2, 512], F32, tag="wa", name=f"wa{ps_idx}")
            for kt in range(KT):
                T = psum.tile([128, 4, 512], F32, tag="s", name=f"s{ps_idx}_{kt}")
                for hh in range(4):
                    strip = hh * 32
                    nc.tensor.matmul(
                        T[:, hh, :],
                        lhsT=kT_sb[strip:strip + 32, half,
                                   kt * 128:(kt + 1) * 128],
                        rhs=qT_sb[strip:strip + 32, half,
                                  qhalf * 512:(qhalf + 1) * 512],
                        start=True, stop=True,
                        tile_position=(strip, 0))
                if ps_idx == 1 and kt % 2 == 1 and proj1:
                    proj1.pop(0)()
                if kt == 2 and pending_fin:
                    pending_fin.pop(0)()       # cast: frees the wa slot
                elif kt >= 3:
                    for _ in range(min(2, len(pending_fin))):
                        pending_fin.pop(0)()
                es = es_pool.tile([128, 2048], F16, tag="es")
                nc.scalar.activation(es, T.rearrange("p h f -> p (h f)"),
                                     mybir.ActivationFunctionType.Exp)
                eb = eb_tiles[ps_idx * 4 + kt // 2]
                p = p_pool.tile([128, 2048], F16, tag="p")
                mul_eng = nc.gpsimd if (kt % 4 == 3 and ps_idx < 3) else nc.vector
                mul_eng.tensor_mul(out=p, in0=es, in1=eb[:, kt % 2, :])

                def emit_pv(wa=wa, p=p, kt=kt, quad=quad):
                    for pi in range(2):
                        for hh2 in range(2):
                            h = quad * 4 + pi * 2 + hh2
                            cstrip = hh2 * 64
                            nc.tensor.matmul(
                                wa[cstrip:cstrip + 33, pi, :],
                                lhsT=v_sb[:, kt, h, :],
                                rhs=p[:, (pi * 2 + hh2) * 512:
                                      (pi * 2 + hh2 + 1) * 512],
                                start=(kt == 0), stop=(kt == KT - 1),
                                tile_position=(0, cstrip))
                pv_queue.append(emit_pv)
                if len(pv_queue) > 2:
                    pv_queue.pop(0)()
                if kt % 2 == 1:
                    g_next = ps_idx * 4 + kt // 2 + 4
                    if g_next < NCHUNK:
                        fetch_eb(g_next)
            while len(pending_fin) > 0:
                pending_fin.pop(0)()
            pending_fin = make_finalizer(ps_idx, wa)
        while pv_queue:
            pv_queue.pop(0)()
        while pending_fin:
            pending_fin.pop(0)()

    _split_multi_waits(nc)
    return nc


_NC = None


def _get_nc():
    global _NC
    if _NC is None:
        _NC = build()
    return _NC


def _make_in_maps(q_data, m_data, batched_bias, query_w, query_b, key_w,
                  value_w, gating_w):
    q_data = np.asarray(q_data, dtype=np.float32)
    m_data = np.asarray(m_data, dtype=np.float32)
    batched_bias = np.asarray(batched_bias, dtype=np.float32)
    wq = np.ascontiguousarray(np.asarray(query_w, np.float32).reshape(A, HV)).astype(np.float16)
    wk = np.ascontiguousarray(np.asarray(key_w, np.float32).reshape(A, HV)).astype(np.float16)
    wv = np.ascontiguousarray(np.asarray(value_w, np.float32).reshape(A, HV)).astype(np.float16)
    wg = np.ascontiguousarray(np.asarray(gating_w, np.float32).reshape(A, HV)).astype(np.float16)
    bq = np.ascontiguousarray(
        (np.asarray(query_b, np.float32) * KEY_SCALE).reshape(HV))
    in_maps = []
    for b in range(N_CORES):
        # eb[(quad, qhalf), kt, k-row, (hh, q512)] = exp(bias)/16
        eb = (np.exp(batched_bias[b]) * EB_SCALE).astype(np.float16)  # [h, q, k]
        eb = eb.transpose(0, 2, 1)                    # [h, k, q]
        eb = eb.reshape(2, 4, K, 2, 512)              # [quad, hh, k, qhalf, 512]
        eb = eb.transpose(0, 3, 2, 1, 4)              # [quad, qhalf, k, hh, 512]
        eb = np.ascontiguousarray(eb).reshape(NPASS, KT, 128, 2048)
        in_maps.append({
            "qT": np.ascontiguousarray(q_data[b].T).astype(np.float16),
            "mT": np.ascontiguousarray(m_data[b].T).astype(np.float16),
            "expb": eb,
            "wq": wq, "wk": wk, "wv": wv, "wg": wg, "bq": bq,
        })
    return in_maps


def run_spmd(in_maps, **kw):
    nc = _get_nc()
    return run_bass_kernel_spmd(nc, in_maps, list(range(N_CORES)), **kw)


def kernel(q_data, m_data, batched_bias, query_w, query_b, key_w, value_w,
           gating_w):
    in_maps = _make_in_maps(q_data, m_data, batched_bias, query_w, query_b,
                            key_w, value_w, gating_w)
    res = run_spmd(in_maps)
    out = np.stack([res.results[b]["out"] for b in range(N_CORES)])
    return out.reshape(B, S, H, V).astype(np.float32)


# revision 11
# speedup vs baseline: 1.1220x; 1.1220x over previous
"""CrossAttention Trainium2 Bass kernel — 8 cores, batch-per-core sharding.

Per core b: all H=8 heads of batch b, processed as 4 passes of
(head-QUAD x q-half).  Within a pass, per kt:
  sT[k, (h4, q512)] = k @ qT    4 heads row-strip packed (strips 0/32/64/96)
  es = exp(sT)                  ONE activation over [128, 2048]
  p  = es * exp(bias)           host-precomputed eb fp16 (scaled 1/16)
  waT += v' @ p                 4 heads col-strip packed into 2 psum banks

The PE stream is software-pipelined with a 2-step skew (qk(kt) issues
before pv(kt-2)) so mul/pv latency never blocks the exp->qk->exp chain,
which paces the kernel at ~2.8us per kt (scalar exp 1.97us + qk WAR).
Finalize (cast fp16, PE-transpose, recip(den), out = waT*r*gate) is
interleaved into the next pass.  gate = sigmoid(q_data @ Wg) runs first
so the ACT table set switches exactly once (sigmoid set -> exp set).
Half-1 q/k projections run as 1-bank PSUM installments during pass 1.

PSUM (8 banks): logits 4 + pv accum 2 + transpose staging 1 + gate/proj 1.
exp(bias) rides two DMA queues (sync + gpsimd) as contiguous 1MB chunks.
"""
import numpy as np
from contextlib import ExitStack

import concourse.bass as bass
import concourse.tile as tile
from concourse import mybir
from concourse.bass import AP
from concourse.bass_utils import run_bass_kernel_spmd
from concourse.masks import make_identity

F32 = mybir.dt.float32
F16 = mybir.dt.float16

B, S, K, H, C, V, A = 8, 1024, 1024, 8, 32, 32, 256
HV = H * V            # 256
KEY_SCALE = C ** -0.5
N_CORES = 8
QT = S // 128         # 8 q tiles
KT = K // 128         # 8 k tiles
NPASS = 4             # (quad, q-half) passes
NCHUNK = 16           # eb chunks of [128, 2, 2048]
EB_SCALE = 1.0 / 16.0  # host scales exp(bias); cancels in softmax


def _split_multi_waits(nc, max_waits=1):
    """walrus in this container allows only one semaphore wait per
    instruction; hoist extras onto same-engine nops inserted just before."""
    ctr = 0
    for fn in nc.m.functions:
        for blk in fn.blocks:
            insts = list(blk.instructions)
            out = []
            changed = False
            for inst in insts:
                si = inst.sync_info
                waits = list(si.on_wait) if (si is not None and si.on_wait) else []
                if len(waits) > max_waits:
                    changed = True
                    extra, keep = waits[:-max_waits], waits[-max_waits:]
                    for w in extra:
                        ctr += 1
                        nop = mybir.InstNoOp(
                            name=f"waitsplit_{ctr}",
                            engine=inst.engine,
                            ins=[],
                            outs=[],
                            sync_info=mybir.SyncInfo(on_wait=[w], on_update=[]),
                            bass_nofuse=True,
                        )
                        out.append(nop)
                    si.on_wait = keep
                out.append(inst)
            if changed:
                if hasattr(blk, "set_instructions"):
                    blk.set_instructions(out)
                else:
                    blk.instructions = out
    return ctr


def build():
    nc = bass.Bass()
    qT_d = nc.declare_dram_parameter("qT", [A, S], F16, isOutput=False)
    mT_d = nc.declare_dram_parameter("mT", [A, K], F16, isOutput=False)
    expb_d = nc.declare_dram_parameter("expb", [NPASS, KT, 128, 2048], F16,
                                       isOutput=False)
    wq_d = nc.declare_dram_parameter("wq", [A, HV], F16, isOutput=False)
    wk_d = nc.declare_dram_parameter("wk", [A, HV], F16, isOutput=False)
    wv_d = nc.declare_dram_parameter("wv", [A, HV], F16, isOutput=False)
    wg_d = nc.declare_dram_parameter("wg", [A, HV], F16, isOutput=False)
    bq_d = nc.declare_dram_parameter("bq", [HV], F32, isOutput=False)
    out_d = nc.declare_dram_parameter("out", [S, HV], F32, isOutput=True)

    with tile.TileContext(nc) as tc, ExitStack() as ctx:
        singles = ctx.enter_context(tc.tile_pool(name="singles", bufs=1))
        eb_pool = ctx.enter_context(tc.tile_pool(name="eb", bufs=4))
        es_pool = ctx.enter_context(tc.tile_pool(name="es", bufs=3))
        p_pool = ctx.enter_context(tc.tile_pool(name="pp", bufs=3))
        fin_pool = ctx.enter_context(tc.tile_pool(name="fin", bufs=2))
        dr_pool = ctx.enter_context(tc.tile_pool(name="dr", bufs=2))
        rg_pool = ctx.enter_context(tc.tile_pool(name="rg", bufs=2))
        psum = ctx.enter_context(tc.tile_pool(name="ps", bufs=1, space="PSUM"))

        # ---------- phase 0: loads split across both DMA queues ------------
        qraw = singles.tile([128, 2, S], F16)       # [a-chunk part, chunk, q]
        mraw = singles.tile([128, 2, K], F16)
        wq_sb = singles.tile([128, 2, HV], F16)
        wk_sb = singles.tile([128, 2, HV], F16)
        wv_sb = singles.tile([128, 2, HV], F16)
        wg_sb = singles.tile([128, 2, HV], F16)
        bq_sb = singles.tile([128, 2], F32)
        # sync queue: gate+q-proj inputs first
        for ac in range(2):
            nc.sync.dma_start(out=qraw[:, ac, :], in_=qT_d[ac * 128:(ac + 1) * 128, :])
        for ac in range(2):
            nc.sync.dma_start(out=wg_sb[:, ac, :], in_=wg_d[ac * 128:(ac + 1) * 128, :])
        for ac in range(2):
            nc.sync.dma_start(out=wq_sb[:, ac, :], in_=wq_d[ac * 128:(ac + 1) * 128, :])
        nc.sync.dma_start(out=bq_sb, in_=bq_d.rearrange("(h p) -> p h", p=128))
        # gpsimd queue: k/v-proj inputs, then the eb chunk stream
        for ac in range(2):
            nc.gpsimd.dma_start(out=mraw[:, ac, :], in_=mT_d[ac * 128:(ac + 1) * 128, :])
        for ac in range(2):
            nc.gpsimd.dma_start(out=wk_sb[:, ac, :], in_=wk_d[ac * 128:(ac + 1) * 128, :])
        for ac in range(2):
            nc.gpsimd.dma_start(out=wv_sb[:, ac, :], in_=wv_d[ac * 128:(ac + 1) * 128, :])
        ident = singles.tile([128, 128], F16)
        make_identity(nc, ident)
        v_sb = singles.tile([128, KT, H, V + 1], F16)
        nc.gpsimd.memset(v_sb, 1.0)

        # eb chunks: chunk g covers (pass g//4, kts 2*(g%4)..2*(g%4)+1)
        eb_tiles = {}

        def fetch_eb(g):
            ps, c = g // 4, g % 4
            t = eb_pool.tile([128, 2, 2048], F16, tag="eb", name=f"eb{ps}_{c}")
            eng = nc.gpsimd if (g < 3 or g % 2 == 0) else nc.sync
            eng.dma_start(
                out=t,
                in_=expb_d[ps, 2 * c:2 * c + 2, :, :].rearrange("k p f -> p k f"))
            eb_tiles[g] = t

        for g in range(4):
            fetch_eb(g)

        # ---------- phase 1: gate + half-0 projections ----------
        # gate first: its sigmoids run before any exp -> one table switch.
        gate_sb = singles.tile([128, QT, HV], F16)
        for grp in range(4):         # qt groups of 2, 1 psum bank each
            ps_g = psum.tile([128, 2, HV], F32, tag="g", name=f"ps_projg{grp}")
            for qq in range(2):
                qt = grp * 2 + qq
                for ac in range(2):
                    nc.tensor.matmul(ps_g[:, qq, :],
                                     lhsT=qraw[:, ac, qt * 128:(qt + 1) * 128],
                                     rhs=wg_sb[:, ac, :], start=(ac == 0),
                                     stop=(ac == 1))
            nc.scalar.activation(gate_sb[:, grp * 2:(grp + 1) * 2, :], ps_g,
                                 mybir.ActivationFunctionType.Sigmoid)

        qT_sb = singles.tile([128, 2, S], F16)
        kT_sb = singles.tile([128, 2, K], F16)

        def emit_qproj(half, qh, ptag):
            t = psum.tile([128, 512] if ptag == "g" else [128, 2, 512],
                          F32, tag=ptag, name=f"q{half}_{qh}")
            dst = t if ptag == "g" else t[:, qh, :]
            for ac in range(2):
                nc.tensor.matmul(dst if ptag == "g" else t[:, qh, :],
                                 lhsT=wq_sb[:, ac, half * 128:(half + 1) * 128],
                                 rhs=qraw[:, ac, qh * 512:(qh + 1) * 512],
                                 start=(ac == 0), stop=(ac == 1))
            nc.vector.tensor_scalar(
                qT_sb[:, half, qh * 512:(qh + 1) * 512],
                dst, KEY_SCALE, bq_sb[:, half:half + 1],
                mybir.AluOpType.mult, mybir.AluOpType.add)

        def emit_kproj(half, qh, ptag):
            t = psum.tile([128, 512] if ptag == "g" else [128, 2, 512],
                          F32, tag=ptag, name=f"k{half}_{qh}")
            dst = t if ptag == "g" else t[:, qh, :]
            for ac in range(2):
                nc.tensor.matmul(dst,
                                 lhsT=wk_sb[:, ac, half * 128:(half + 1) * 128],
                                 rhs=mraw[:, ac, qh * 512:(qh + 1) * 512],
                                 start=(ac == 0), stop=(ac == 1))
            nc.vector.tensor_copy(out=kT_sb[:, half, qh * 512:(qh + 1) * 512],
                                  in_=dst)

        # half 0 up-front: q on the "s" slot, k on the "wa" slot (decoupled)
        ps_q0 = psum.tile([128, 2, 512], F32, tag="s", name="ps_q0")
        for qh in range(2):
            for ac in range(2):
                nc.tensor.matmul(ps_q0[:, qh, :],
                                 lhsT=wq_sb[:, ac, 0:128],
                                 rhs=qraw[:, ac, qh * 512:(qh + 1) * 512],
                                 start=(ac == 0), stop=(ac == 1))
        nc.vector.tensor_scalar(
            qT_sb[:, 0, :], ps_q0.rearrange("p a f -> p (a f)"),
            KEY_SCALE, bq_sb[:, 0:1],
            mybir.AluOpType.mult, mybir.AluOpType.add)
        ps_k0 = psum.tile([128, 2, 512], F32, tag="wa", name="ps_k0")
        for qh in range(2):
            for ac in range(2):
                nc.tensor.matmul(ps_k0[:, qh, :],
                                 lhsT=wk_sb[:, ac, 0:128],
                                 rhs=mraw[:, ac, qh * 512:(qh + 1) * 512],
                                 start=(ac == 0), stop=(ac == 1))
        nc.vector.tensor_copy(out=kT_sb[:, 0, :],
                              in_=ps_k0.rearrange("p a f -> p (a f)"))

        # v natural layout + ones column: [k-tile part, kt, h, v+1] fp16
        def emit_vproj(grp):
            ps_v = psum.tile([128, 4, HV], F32, tag="wa", name=f"ps_projv{grp}")
            for kq in range(4):
                kt = grp * 4 + kq
                for ac in range(2):
                    nc.tensor.matmul(ps_v[:, kq, :],
                                     lhsT=mraw[:, ac, kt * 128:(kt + 1) * 128],
                                     rhs=wv_sb[:, ac, :], start=(ac == 0),
                                     stop=(ac == 1))
            nc.vector.tensor_copy(
                out=v_sb[:, grp * 4:(grp + 1) * 4, :, 0:V],
                in_=ps_v.rearrange("p k (h c) -> p k h c", c=V))

        # ---------- phase 2: quad passes, 2-step qk/pv skew ----------
        # (v projections are emitted inside pass 0's kt0/kt1 PE slots)
        out_sb = singles.tile([128, QT, HV], F32)
        pending_fin = []   # finalize steps of the previous pass
        pv_queue = []      # deferred pv emitters

        def make_finalizer(ps_idx, wa):
            quad, qhalf = ps_idx // 2, ps_idx % 2
            fin = fin_pool.tile([128, 1024], F16, tag="fin", name=f"fin{ps_idx}")
            ps_t = psum.tile([128, 8, 128], F16, tag="t", name=f"pst{ps_idx}")
            steps = [lambda: nc.vector.tensor_copy(
                out=fin.rearrange("p (k f) -> p k f", k=2), in_=wa)]
            for ch in range(8):      # chunk = pi*4 + j
                steps.append(lambda ch=ch: nc.tensor.transpose(
                    ps_t[:, ch, :], fin[:, ch * 128:(ch + 1) * 128], ident))

            def tail():
                # den at ps_t[:, pi*4+j, 32 + 64*hh] -> d[128, j, head(pi,hh)]
                d_src = AP(ps_t.tensor, ps_t[:, 0, 32].offset,
                           [list(ps_t.ap)[0], [128, 4], [512, 2], [64, 2]])
                d_sb = dr_pool.tile([128, 4, 4], F32, tag="d", name=f"d{ps_idx}")
                nc.vector.tensor_copy(out=d_sb, in_=d_src)
                r_sb = dr_pool.tile([128, 4, 4], F32, tag="r", name=f"r{ps_idx}")
                nc.vector.reciprocal(out=r_sb, in_=d_sb)
                # rg[q, j, head, v] = gate * r (r broadcast over v, stride 0)
                r_b = AP(r_sb.tensor, r_sb.offset,
                         [list(r_sb.ap)[0], [4, 4], [1, 4], [0, V]])
                rg = rg_pool.tile([128, 4, 4, V], F32, tag="rg",
                                  name=f"rg{ps_idx}")
                nc.vector.tensor_mul(
                    out=rg,
                    in0=gate_sb[:, qhalf * 4:(qhalf + 1) * 4,
                                quad * 128:(quad + 1) * 128].rearrange(
                        "p j (h v) -> p j h v", v=V),
                    in1=r_b)
                # out[q, j, quad cols] = waT * rg   (one op per pair pi)
                for pi in range(2):
                    src = AP(ps_t.tensor, ps_t[:, pi * 4, 0].offset,
                             [list(ps_t.ap)[0], [128, 4], [64, 2], [1, V]])
                    nc.vector.tensor_mul(
                        out=out_sb[:, qhalf * 4:(qhalf + 1) * 4,
                                   quad * 128 + pi * 64:
                                   quad * 128 + (pi + 1) * 64].rearrange(
                            "p j (k v) -> p j k v", v=V),
                        in0=src,
                        in1=rg[:, :, pi * 2:(pi + 1) * 2, :])

            def store():
                for j in range(4):
                    qt = qhalf * 4 + j
                    nc.sync.dma_start(
                        out=out_d[qt * 128:(qt + 1) * 128,
                                  quad * 128:(quad + 1) * 128],
                        in_=out_sb[:, qt, quad * 128:(quad + 1) * 128])
            steps.append(tail)
            steps.append(store)
            return steps

        # half-1 projection installments, spread through pass 1 (g slot)
        proj1 = [lambda qh=qh: emit_qproj(1, qh, "g") for qh in range(2)]
        proj1 += [lambda qh=qh: emit_kproj(1, qh, "g") for qh in range(2)]

        for ps_idx in range(NPASS):
            quad, qhalf = ps_idx // 2, ps_idx % 2
            half = quad
            wa_box = [None]

            def get_wa(wa_box=wa_box, ps_idx=ps_idx):
                if wa_box[0] is None:
                    wa_box[0] = psum.tile([128, 2, 512], F32, tag="wa",
                                          name=f"wa{ps_idx}")
                return wa_box[0]

            for kt in range(KT):
                T = psum.tile([128, 4, 512], F32, tag="s", name=f"s{ps_idx}_{kt}")
                for hh in range(4):
                    strip = hh * 32
                    nc.tensor.matmul(
                        T[:, hh, :],
                        lhsT=kT_sb[strip:strip + 32, half,
                                   kt * 128:(kt + 1) * 128],
                        rhs=qT_sb[strip:strip + 32, half,
                                  qhalf * 512:(qhalf + 1) * 512],
                        start=True, stop=True,
                        tile_position=(strip, 0))
                if ps_idx == 0 and kt < 2:
                    emit_vproj(kt)
                if ps_idx == 1 and kt % 2 == 1 and proj1:
                    proj1.pop(0)()
                if kt >= 3:
                    for _ in range(min(2, len(pending_fin))):
                        pending_fin.pop(0)()
                es = es_pool.tile([128, 2048], F16, tag="es")
                nc.scalar.activation(es, T.rearrange("p h f -> p (h f)"),
                                     mybir.ActivationFunctionType.Exp)
                eb = eb_tiles[ps_idx * 4 + kt // 2]
                p = p_pool.tile([128, 2048], F16, tag="p")
                nc.vector.tensor_mul(out=p, in0=es, in1=eb[:, kt % 2, :])

                def emit_pv(get_wa=get_wa, p=p, kt=kt, quad=quad):
                    wa = get_wa()
                    for pi in range(2):
                        for hh2 in range(2):
                            h = quad * 4 + pi * 2 + hh2
                            cstrip = hh2 * 64
                            nc.tensor.matmul(
                                wa[cstrip:cstrip + 33, pi, :],
                                lhsT=v_sb[:, kt, h, :],
                                rhs=p[:, (pi * 2 + hh2) * 512:
                                      (pi * 2 + hh2 + 1) * 512],
                                start=(kt == 0), stop=(kt == KT - 1),
                                tile_position=(0, cstrip))
                pv_queue.append(emit_pv)
                if len(pv_queue) > 2:
                    pv_queue.pop(0)()
                if kt == 1 and pending_fin:
                    # cast: frees the wa slot; must be emitted AFTER the
                    # pv(kt7) pop above (Tile orders deps by emission).
                    pending_fin.pop(0)()
                if kt % 2 == 1:
                    g_next = ps_idx * 4 + kt // 2 + 4
                    if g_next < NCHUNK:
                        fetch_eb(g_next)
            while len(pending_fin) > 0:
                pending_fin.pop(0)()
            pending_fin = make_finalizer(ps_idx, get_wa())
        while pv_queue:
            pv_queue.pop(0)()
        while pending_fin:
            pending_fin.pop(0)()

    _split_multi_waits(nc)
    return nc


_NC = None


def _get_nc():
    global _NC
    if _NC is None:
        _NC = build()
    return _NC


def _make_in_maps(q_data, m_data, batched_bias, query_w, query_b, key_w,
                  value_w, gating_w):
    q_data = np.asarray(q_data, dtype=np.float32)
    m_data = np.asarray(m_data, dtype=np.float32)
    batched_bias = np.asarray(batched_bias, dtype=np.float32)
    wq = np.ascontiguousarray(np.asarray(query_w, np.float32).reshape(A, HV)).astype(np.float16)
    wk = np.ascontiguousarray(np.asarray(key_w, np.float32).reshape(A, HV)).astype(np.float16)
    wv = np.ascontiguousarray(np.asarray(value_w, np.float32).reshape(A, HV)).astype(np.float16)
    wg = np.ascontiguousarray(np.asarray(gating_w, np.float32).reshape(A, HV)).astype(np.float16)
    bq = np.ascontiguousarray(
        (np.asarray(query_b, np.float32) * KEY_SCALE).reshape(HV))
    in_maps = []
    for b in range(N_CORES):
        # eb[(quad, qhalf), kt, k-row, (hh, q512)] = exp(bias)/16
        eb = (np.exp(batched_bias[b]) * EB_SCALE).astype(np.float16)  # [h, q, k]
        eb = eb.transpose(0, 2, 1)                    # [h, k, q]
        eb = eb.reshape(2, 4, K, 2, 512)              # [quad, hh, k, qhalf, 512]
        eb = eb.transpose(0, 3, 2, 1, 4)              # [quad, qhalf, k, hh, 512]
        eb = np.ascontiguousarray(eb).reshape(NPASS, KT, 128, 2048)
        in_maps.append({
            "qT": np.ascontiguousarray(q_data[b].T).astype(np.float16),
            "mT": np.ascontiguousarray(m_data[b].T).astype(np.float16),
            "expb": eb,
            "wq": wq, "wk": wk, "wv": wv, "wg": wg, "bq": bq,
        })
    return in_maps


def run_spmd(in_maps, **kw):
    nc = _get_nc()
    return run_bass_kernel_spmd(nc, in_maps, list(range(N_CORES)), **kw)


def kernel(q_data, m_data, batched_bias, query_w, query_b, key_w, value_w,
           gating_w):
    in_maps = _make_in_maps(q_data, m_data, batched_bias, query_w, query_b,
                            key_w, value_w, gating_w)
    res = run_spmd(in_maps)
    out = np.stack([res.results[b]["out"] for b in range(N_CORES)])
    return out.reshape(B, S, H, V).astype(np.float32)
